# revision 13
# baseline (speedup 1.0000x reference)
"""Joint soft-histogram kernel for Trainium2 (Bass/Tile), 8-core data parallel.

Math (per batch b, K=256, L=1/256, W=L/2.5, N=65536 pixels):
    phi_k(x) = sigmoid((x - k*L)/W) - sigmoid((x - (k+1)*L)/W)
             = S_k(x) - S_{k+1}(x),   S_k(x) = sigmoid(640*x - 2.5*k)
    out[k, j] = sum_n phi_k(x_n) * phi_j(y_n) / N

Half-telescope: out[k, j] = (M[k, j] - M[k, j+1]) / N with M = Phi_x^T @ T_y,
T_y[n, j] = S_j(y_n), j = 0..256. Only the x side needs the adjacent
difference before the matmul; the y-side difference collapses onto the tiny
(256, 257) M. M entries stay O(600), so fp32 PSUM accumulation is safe, and
fp16 Phi/T operands give ~3e-4 relative error overall.

v2 engine plan (vs v1 which spent 484us in per-chunk DVE TENSOR_SCALARs and
338us in GPSIMD diffs):
  - pre-activation A[p, c*KP+j] = 640*v[p,c] - 2.5*j is an outer sum. Two ways:
      DVE: ONE tensor_tensor per 16-chunk group with broadcast access
           patterns (in0 = pixel col, inner stride 0; in1 = static kr row,
           outer stride 0). 1x mode, ~4.4us/group, no per-chunk instructions.
      PE:  per chunk, rank-2 matmul  [ones; 640*v-transposed]^T @ [kr; ones]
           -> PSUM. The transposed pixel operand comes precomputed from the
           host (layout prep). ~217ns/chunk, overlaps DVE/ACT.
  - sigmoid: big staged ACTIVATEs (1 elem/cyc/lane floor, ~238us total; this
    is the dense-algorithm floor and sets the target wall time).
  - x-side adjacent diff: DVE grouped TT fp16 (GPSIMD was 7x slower).
  - The DVE/PE preadd split is tunable (X_ENG/Y_ENG) to balance engines.

Sharding: pure data parallel, batch b -> core b.
"""

import numpy as np

import concourse.bass as bass
import concourse.tile as tile
from concourse import bacc, mybir
from concourse.bass_utils import run_bass_kernel_spmd

F32 = mybir.dt.float32
F16 = mybir.dt.float16

B = 8
K = 256
KB = K + 1            # 257 taps on the y/T axis (j = 0..256)
KX = K + 1            # 257 taps on the x/S axis (k = 0..256; phi needs 0..255)
KP = K + 2            # 258: per-chunk stride in staged tiles (even)
NPIX = 65536
NCHUNK = 512
XG = 16               # chunks per staged group
NG = NCHUNK // XG     # 32 groups
GF = XG * KP          # staged group free size (4128)
INV_N = 1.0 / NPIX
YSTAGE = 2            # y chunks per PSUM preadd stage (PE path)

# --- tuning knobs -----------------------------------------------------------
# Per x-group: preadd engine 'v' (DVE broadcast-TT) or 'p' (PE rank-2 matmuls)
X_ENG = ['v' if g % 3 != 2 else 'p' for g in range(NG)]
# Per y-stage (NCHUNK // YSTAGE stages): 'p' (PE) or 'v' (DVE broadcast-TT)
NYST = NCHUNK // YSTAGE
Y_ENG = ['p'] * NYST
# ---------------------------------------------------------------------------

_cached_nc = None


def _build():
    nc = bacc.Bacc("TRN2")
    xd = nc.declare_dram_parameter("x", [128, 512], F32, isOutput=False)
    # wx/wy: [2, 65536]; row 0 = ones, row 1 = 640*val in chunk-major
    # (transposed) order: wx[1, c*128 + p] = 640*x[p, c]. Streamed per group.
    wxd = nc.declare_dram_parameter("wx", [2, 65536], F32, isOutput=False)
    wyd = nc.declare_dram_parameter("wy", [2, 65536], F32, isOutput=False)
    kd = nc.declare_dram_parameter("krow", [128, KP], F32, isOutput=False)
    # kr2x: rhs for PE preadd, x flavor: [[-2.5*j]*KX; [1]*KX]
    k2xd = nc.declare_dram_parameter("kr2x", [2, KX], F32, isOutput=False)
    k2yd = nc.declare_dram_parameter("kr2y", [2, KB], F32, isOutput=False)
    od = nc.declare_dram_parameter("out", [256, 256], F32, isOutput=True)

    sig = mybir.ActivationFunctionType.Sigmoid
    add = mybir.AluOpType.add

    with tile.TileContext(nc) as tc:
        with (
            tc.tile_pool(name="singles", bufs=1) as singles,
            tc.tile_pool(name="stage32", bufs=2) as stage32,
            tc.tile_pool(name="stage16", bufs=3) as stage16,
            tc.tile_pool(name="ty16", bufs=4) as ty16,
            tc.tile_pool(name="work", bufs=4) as work,
            tc.tile_pool(name="wpool", bufs=3) as wpool,
            tc.tile_pool(name="psum", bufs=1, space="PSUM") as psum,
            tc.tile_pool(name="psum_ay", bufs=2, space="PSUM") as psum_ay,
        ):
            xt = singles.tile([128, 512], F32)
            nc.sync.dma_start(out=xt, in_=xd[:, :])
            kr = singles.tile([128, KP], F32)
            nc.sync.dma_start(out=kr, in_=kd[:, :])
            k2x = singles.tile([2, KX], F32)
            nc.sync.dma_start(out=k2x, in_=k2xd[:, :])
            k2y = singles.tile([2, KB], F32)
            nc.sync.dma_start(out=k2y, in_=k2yd[:, :])
            GW = XG * 128  # W cols per group

            def w_group(wd, g, tag):
                t = wpool.tile([2, GW], F32, tag=tag)
                nc.sync.dma_start(out=t, in_=wd[:, g * GW:(g + 1) * GW])
                return t

            M = psum.tile([128, 2, 512], F32)

            # ---- x side: staged preadd (DVE bcast-TT or PE) + big sigmoid
            # + grouped diff -> phi(fp16)
            def x_group(g):
                c0 = g * XG
                if X_ENG[g] == 'v':
                    wxg = None
                    ax = stage32.tile([128, XG, KP], F32, tag="ax")
                    nc.vector.tensor_tensor(
                        out=ax,
                        in0=xt[:, c0:c0 + XG].unsqueeze(2)
                            .broadcast_to([128, XG, KP]),
                        in1=kr.unsqueeze(1).broadcast_to([128, XG, KP]),
                        op=add,
                    )
                    sx = stage16.tile([128, XG, KP], F16, tag="sx")
                    nc.scalar.activation(out=sx, in_=ax, func=sig)
                else:
                    # PE path: 16 rank-2 matmuls into a PSUM stage of 4
                    # chunks each, sigmoid from PSUM.
                    wxg = w_group(wxd, g, "wxg")
                    sx = stage16.tile([128, XG, KP], F16, tag="sx")
                    for st in range(XG // YSTAGE):
                        axp = psum_ay.tile([128, YSTAGE, 512], F32, tag="aprep")
                        for i in range(YSTAGE):
                            lc = st * YSTAGE + i
                            nc.tensor.matmul(
                                axp[:, i, 0:KX],
                                lhsT=wxg[:, lc * 128:lc * 128 + 128],
                                rhs=k2x,
                                start=True,
                                stop=True,
                            )
                        nc.scalar.activation(
                            out=sx[:, st * YSTAGE:(st + 1) * YSTAGE, 0:KX],
                            in_=axp[:, :, 0:KX], func=sig,
                        )
                ph = stage16.tile([128, GF], F16, tag="ph")
                sxf = sx.rearrange("p a b -> p (a b)")
                nc.vector.tensor_sub(
                    out=ph[:, 0:GF - 1], in0=sxf[:, 0:GF - 1], in1=sxf[:, 1:GF],
                )
                return ph

            # ---- y side: T_y = sigmoid(640*y - 2.5*j), j=0..256, per chunk
            def y_stage(st, wyg, g):
                c0 = st * YSTAGE
                ty = ty16.tile([128, YSTAGE, KB], F16, tag="ty")
                ayp = psum_ay.tile([128, YSTAGE, 512], F32, tag="aprep")
                for i in range(YSTAGE):
                    lc = c0 + i - g * XG
                    nc.tensor.matmul(
                        ayp[:, i, 0:KB],
                        lhsT=wyg[:, lc * 128:lc * 128 + 128],
                        rhs=k2y,
                        start=True,
                        stop=True,
                    )
                nc.scalar.activation(out=ty, in_=ayp[:, :, 0:KB], func=sig)
                return ty

            # ---- main loop: interleave x groups, y stages, matmuls
            for g in range(NG):
                ph = x_group(g)
                wyg = w_group(wyd, g, "wyg")
                for st in range(g * XG // YSTAGE, (g + 1) * XG // YSTAGE):
                    ty = y_stage(st, wyg, g)
                    for i in range(YSTAGE):
                        c = st * YSTAGE + i
                        first = c == 0
                        last = c == NCHUNK - 1
                        ic = c - g * XG
                        nc.tensor.matmul(
                            M[:, 0, 0:KB],
                            lhsT=ph[:, ic * KP: ic * KP + 128],
                            rhs=ty[:, i, :],
                            start=first,
                            stop=last,
                        )
                        nc.tensor.matmul(
                            M[:, 1, 0:KB],
                            lhsT=ph[:, ic * KP + 128: ic * KP + 256],
                            rhs=ty[:, i, :],
                            start=first,
                            stop=last,
                        )

            for h in range(2):
                t1 = work.tile([128, KB], F32, tag="ep")
                nc.scalar.activation(
                    out=t1,
                    in_=M[:, h, 0:KB],
                    func=mybir.ActivationFunctionType.Copy,
                    scale=INV_N,
                )
                t2 = work.tile([128, K], F32, tag="ep2")
                nc.vector.tensor_sub(out=t2, in0=t1[:, 0:K], in1=t1[:, 1:KB])
                nc.sync.dma_start(out=od[128 * h: 128 * (h + 1), :], in_=t2)

    nc.finalize()
    return nc


def _get_nc():
    global _cached_nc
    if _cached_nc is None:
        _cached_nc = _build()
    return _cached_nc


def _krow():
    row = np.arange(KP, dtype=np.float32) * np.float32(-2.5)
    return np.tile(row[None, :], (128, 1))


def _kr2(n):
    return np.stack([
        np.arange(n, dtype=np.float32) * np.float32(-2.5),
        np.ones(n, dtype=np.float32),
    ])


def _wpair(v640):
    # v640: (128, 512) scaled values -> [2, 65536] chunk-major with ones row
    flat = np.ascontiguousarray(v640.T).reshape(-1)  # c*128 + p
    return np.ascontiguousarray(
        np.stack([np.ones(NPIX, dtype=np.float32), flat]))


def _in_maps(x, y):
    x = np.asarray(x, dtype=np.float32)
    y = np.asarray(y, dtype=np.float32)
    kr = _krow()
    k2x = _kr2(KX)
    k2y = _kr2(KB)
    maps = []
    for b in range(B):
        x6 = (x[b].reshape(128, 512) * np.float32(640.0)).astype(np.float32)
        y6 = (y[b].reshape(128, 512) * np.float32(640.0)).astype(np.float32)
        maps.append({
            "x": np.ascontiguousarray(x6),
            "wx": _wpair(x6), "wy": _wpair(y6),
            "krow": kr, "kr2x": k2x, "kr2y": k2y,
        })
    return maps


def run(x, y, trace=False, **trace_kw):
    """Run on all 8 cores; returns (out (8,256,256) f32, BassKernelResults)."""
    nc = _get_nc()
    res = run_bass_kernel_spmd(nc, _in_maps(x, y), list(range(B)), trace=trace,
                               **trace_kw)
    out = np.stack([res.results[b]["out"] for b in range(B)]).astype(np.float32)
    return out, res


def kernel(x, y):
    out, _ = run(x, y)
    return out


# revision 15
# speedup vs baseline: 3.2425x; 3.2425x over previous
"""Joint soft-histogram kernel for Trainium2 (Bass/Tile), 8-core data parallel.

Math (per batch b, K=256, L=1/256, W=L/2.5, N=65536 pixels):
    phi_k(x) = S_k(x) - S_{k+1}(x),   S_k(x) = sigmoid(640*x - 2.5*k)
    out[k, j] = sum_n phi_k(x_n) * phi_j(y_n) / N

Double telescope: out = Drow(Dcol(M)) / N with M = Sx^T @ Sy (257 x 257),
M[k, j] = sum_n S_k(x_n) * S_j(y_n). Neither side needs a per-chunk adjacent
difference -- both collapse onto the tiny M. M entries grow to O(N), so PSUM
fp32 accumulation is drained to SBUF every SEG chunks (caps entries at
SEG*128 = 8192, keeping roundoff ~1e-3 absolute, ~4e-3 relative after
differencing -- inside the 2e-2 budget).

Engine plan (v3; v1 spent 484us in per-chunk DVE TENSOR_SCALARs + 338us in
GPSIMD diffs; v2's PE rank-2 preadds measured 2.4x slower than modeled):
  - preadd A[p, c*KP+j] = 640*v[p,c] - 2.5*j: ONE broadcast-AP tensor_tensor
    per 16-chunk group (measured 4.4us/group), writing SBUF.
  - sigmoid: one big staged ACTIVATE per group (measured 3.7us/group). This
    ~238us of ACT work is the dense-algorithm floor.
  - a tunable number of preadd groups go to GPSIMD to unload DVE.
  - PE: 2x 128-row matmuls + 1-row tail matmul per chunk, fp16, plus segment
    restarts (start=True zeroes PSUM).

Sharding: pure data parallel, batch b -> core b.
"""

import numpy as np

import concourse.bass as bass
import concourse.tile as tile
from concourse import bacc, mybir
from concourse.bass_utils import run_bass_kernel_spmd

F32 = mybir.dt.float32
F16 = mybir.dt.float16

B = 8
K = 256
KB = K + 1            # 257 sigmoid taps per side (k = 0..256)
KP = K + 2            # 258: per-chunk stride in staged tiles (even)
NPIX = 65536
NCHUNK = 512
XG = 16               # chunks per staged group
NG = NCHUNK // XG     # 32 groups
GF = XG * KP          # staged group free size (4128)
INV_N = 1.0 / NPIX
SEG = 64              # chunks per PSUM accumulation segment
NSEG = NCHUNK // SEG

# --- tuning knobs -----------------------------------------------------------
# Preadd engine per (group, side): 'v' = DVE broadcast-TT, 'g' = GPSIMD TT.
X_ENG = ['v'] * NG
Y_ENG = ['g' if g % 5 == 2 else 'v' for g in range(NG)]
# ---------------------------------------------------------------------------

_cached_nc = None


def _build():
    nc = bacc.Bacc("TRN2")
    xd = nc.declare_dram_parameter("x", [128, 512], F32, isOutput=False)
    yd = nc.declare_dram_parameter("y", [128, 512], F32, isOutput=False)
    kd = nc.declare_dram_parameter("krow", [128, KP], F32, isOutput=False)
    # dmat[k, k'] = [k==k'] - [k==k'+1]; dnext[k, k'] = -[k==0][k'==127]
    dmd = nc.declare_dram_parameter("dmat", [128, 128], F32, isOutput=False)
    dnd = nc.declare_dram_parameter("dnext", [128, 128], F32, isOutput=False)
    od = nc.declare_dram_parameter("out", [256, 256], F32, isOutput=True)

    sig = mybir.ActivationFunctionType.Sigmoid
    add = mybir.AluOpType.add

    with tile.TileContext(nc) as tc:
        with (
            tc.tile_pool(name="singles", bufs=1) as singles,
            tc.tile_pool(name="stage32", bufs=3) as stage32,
            tc.tile_pool(name="stage16", bufs=3) as stage16,
            tc.tile_pool(name="work", bufs=4) as work,
            tc.tile_pool(name="psum", bufs=1, space="PSUM") as psum,
        ):
            xt = singles.tile([128, 512], F32)
            nc.sync.dma_start(out=xt, in_=xd[:, :])
            yt = singles.tile([128, 512], F32)
            nc.sync.dma_start(out=yt, in_=yd[:, :])
            kr = singles.tile([128, KP], F32)
            nc.sync.dma_start(out=kr, in_=kd[:, :])
            dm = singles.tile([128, 128], F32)
            nc.sync.dma_start(out=dm, in_=dmd[:, :])
            dn = singles.tile([128, 128], F32)
            nc.sync.dma_start(out=dn, in_=dnd[:, :])

            # M accumulators in SBUF: rows 0..127 / 128..255 / 256 (tail)
            acc = singles.tile([128, 2, KB], F32)
            acct = singles.tile([128, KB], F32)  # only partition 0 used
            nc.vector.memset(acc, 0.0)
            nc.vector.memset(acct[0:1, :], 0.0)

            # PSUM: M' segment accumulator (rows 0..255 + tail row 256)
            Mp = psum.tile([128, 2, 512], F32)
            Mt = psum.tile([128, 512], F32)  # partition 0 = row 256

            def preadd_sigmoid(src, g, eng, tag):
                a = stage32.tile([128, XG, KP], F32, tag="a" + tag)
                tt = nc.gpsimd.tensor_tensor if eng == 'g' else \
                    nc.vector.tensor_tensor
                tt(
                    out=a,
                    in0=src[:, g * XG:(g + 1) * XG].unsqueeze(2)
                        .broadcast_to([128, XG, KP]),
                    in1=kr.unsqueeze(1).broadcast_to([128, XG, KP]),
                    op=add,
                )
                s = stage16.tile([128, XG, KP], F16, tag="s" + tag)
                nc.scalar.activation(out=s, in_=a, func=sig)
                return s

            for g in range(NG):
                sx = preadd_sigmoid(xt, g, X_ENG[g], "x")
                sy = preadd_sigmoid(yt, g, Y_ENG[g], "y")
                for i in range(XG):
                    c = g * XG + i
                    first = c % SEG == 0
                    last = c % SEG == SEG - 1
                    ty = sy[:, i, 0:KB]
                    nc.tensor.matmul(
                        Mp[:, 0, 0:KB],
                        lhsT=sx[:, i, 0:128],
                        rhs=ty,
                        start=first,
                        stop=last,
                    )
                    nc.tensor.matmul(
                        Mp[:, 1, 0:KB],
                        lhsT=sx[:, i, 128:256],
                        rhs=ty,
                        start=first,
                        stop=last,
                    )
                    nc.tensor.matmul(
                        Mt[0:1, 0:KB],
                        lhsT=sx[:, i, 256:257],
                        rhs=ty,
                        start=first,
                        stop=last,
                    )
                    if last:
                        # drain segment into SBUF accumulators
                        for h in range(2):
                            nc.vector.tensor_add(
                                out=acc[:, h, :], in0=acc[:, h, :],
                                in1=Mp[:, h, 0:KB],
                            )
                        nc.vector.tensor_add(
                            out=acct[0:1, :], in0=acct[0:1, :],
                            in1=Mt[0:1, 0:KB],
                        )

            # Epilogue: out[k, j] = (Mr[k, j] - Mr[k, j+1]) / N with
            # Mr[k, j] = acc[k, j] - acc[k+1, j], row diff via PE:
            # rd_h = dmat^T @ acc_h + dnext^T @ acc_{h+1}.
            for h in range(2):
                rd = psum.tile([128, 512], F32, tag="rd")
                nc.tensor.matmul(
                    rd[:, 0:KB], lhsT=dm, rhs=acc[:, h, :],
                    start=True, stop=False,
                )
                nxt = acc[:, 1, :] if h == 0 else acct[:, :]
                nc.tensor.matmul(
                    rd[:, 0:KB], lhsT=dn, rhs=nxt,
                    start=False, stop=True,
                )
                t1 = work.tile([128, KB], F32, tag="ep")
                nc.scalar.activation(
                    out=t1, in_=rd[:, 0:KB],
                    func=mybir.ActivationFunctionType.Copy, scale=INV_N,
                )
                t2 = work.tile([128, K], F32, tag="ep2")
                nc.vector.tensor_sub(out=t2, in0=t1[:, 0:K], in1=t1[:, 1:KB])
                nc.sync.dma_start(out=od[128 * h: 128 * (h + 1), :], in_=t2)

    nc.finalize()
    return nc


def _get_nc():
    global _cached_nc
    if _cached_nc is None:
        _cached_nc = _build()
    return _cached_nc


def _krow():
    row = np.arange(KP, dtype=np.float32) * np.float32(-2.5)
    return np.tile(row[None, :], (128, 1))


def _dmat():
    d = np.eye(128, dtype=np.float32)
    d -= np.eye(128, k=-1, dtype=np.float32)
    return d


def _dnext():
    d = np.zeros((128, 128), dtype=np.float32)
    d[0, 127] = -1.0
    return d


def _in_maps(x, y):
    x = np.asarray(x, dtype=np.float32)
    y = np.asarray(y, dtype=np.float32)
    kr = _krow()
    maps = []
    for b in range(B):
        x6 = np.ascontiguousarray(x[b].reshape(128, 512) * np.float32(640.0))
        y6 = np.ascontiguousarray(y[b].reshape(128, 512) * np.float32(640.0))
        maps.append({"x": x6, "y": y6, "krow": kr,
                     "dmat": _dmat(), "dnext": _dnext()})
    return maps


def run(x, y, trace=False, **trace_kw):
    """Run on all 8 cores; returns (out (8,256,256) f32, BassKernelResults)."""
    nc = _get_nc()
    res = run_bass_kernel_spmd(nc, _in_maps(x, y), list(range(B)), trace=trace,
                               **trace_kw)
    out = np.stack([res.results[b]["out"] for b in range(B)]).astype(np.float32)
    return out, res


def kernel(x, y):
    out, _ = run(x, y)
    return out
